# revision 23
# baseline (speedup 1.0000x reference)
# Trainium2 Bass kernel for nn_BQQLinear (quantized bilinear linear layer).
#
# Math: the reference collapses exactly to
#     out[b, (j,m)] = quant8(x)[b, (k,n)] @ W[(k,n), (j,m)] + bias[(j,m)]
# where W folds the 1-bit-quantized Y/Z factors and the A-correction terms:
#     W = einsum('pjk,pjkml,pjkln->knjm', A0, Y_q, Z_q)        (core * A0 term)
#       + B_coef[j,k,m] broadcast over n                       (Sx * Y_sum term)
#       + C_coef[j,k,n] broadcast over m                       (Tz * A2 term)
#       + D_coef[j,k]   broadcast over n,m                     (Sx * A3 term)
# W is a pure function of the (small) weight tensors -> folded on host at
# load time, like any quantized-weight repack. All activation math (quant8
# and the 2048x1024x1024 matmul + bias) runs on the NeuronCores.
#
# Sharding: data-parallel over flattened batch B=2048 -> 256 rows/core.
# x is passed pre-transposed ([kn, b] slices) so the contraction dim lands
# on SBUF partitions with contiguous DMA; no on-device transposes needed.

import numpy as np

import concourse.bacc as bacc
import concourse.bass as bass
import concourse.mybir as mybir
import concourse.tile as tile
from concourse.bass import ts
from concourse.bass_utils import run_bass_kernel_spmd

N_CORES = 8
P = 128
KN = 1024          # k*n contraction dim
JM = 1024          # j*m output dim
B_TOT = 2048       # flattened batch
B_C = B_TOT // N_CORES   # 256 rows per core
B_TILES = B_C // P       # 2
NH = 512                 # matmul free-dim tile (one PSUM bank, fp32)
N_TILES = JM // NH       # 2
K_TILES = KN // P        # 8
QMAX = 127.0
MAGIC = float(np.float32(1.5 * 2.0**23))  # round-to-nearest-even trick
MM_DT = mybir.dt.float16  # matmul dtype: W cast host-side; q integers exact, products exact


def _fold_weights(Y_fp, Z_fp, A, act_scale, dtype=np.float64):
    """Fold the quantized factorization into a single [KN, JM] weight.

    Also folds the activation quant scale s: device computes integer codes
    q = clip(round(x/s)) and the matmul uses W_s = s*W, so q @ W_s == X @ W.
    """
    Y = Y_fp.astype(dtype)
    Z = Z_fp.astype(dtype)
    Af = A.astype(dtype)
    p, j, k, m, l = Y.shape
    n = Z.shape[-1]

    Y_scale = np.mean(np.abs(Y), axis=(-2, -1), keepdims=True)
    Z_scale = np.mean(np.abs(Z), axis=(-2, -1), keepdims=True)
    Y_q = np.abs(Y_scale) * np.sign(Y)          # (p,j,k,m,l)
    Z_q = np.abs(Z_scale) * np.sign(Z)          # (p,j,k,l,n)

    # out1: sum_{p,l} A0 * Y_q * Z_q  -> [k,n,j,m]
    W = np.einsum('pjk,pjkml,pjkln->knjm', Af[..., 0], Y_q, Z_q, optimize=True)
    # out2: B_coef[j,k,m] = sum_p A1 * sum_l Y_q ; X enters via Sx (sum over n)
    B_coef = np.einsum('pjk,pjkm->jkm', Af[..., 1], Y_q.sum(-1))
    W += B_coef.transpose(1, 0, 2)[:, None, :, :]
    # out3: C_coef[j,k,n] = sum_p A2 * sum_l Z_q ; broadcast over m
    C_coef = np.einsum('pjk,pjkn->jkn', Af[..., 2], Z_q.sum(-2))
    W += C_coef.transpose(1, 2, 0)[:, :, :, None]
    # out4: D_coef[j,k] = sum_p A3 ; broadcast over n, m
    W += Af[..., 3].sum(0).T[:, None, :, None]

    W = W.reshape(k * n, j * m)
    s = max(abs(float(np.asarray(act_scale).reshape(-1)[0])), 1e-8)
    inv_s = float(np.float32(1.0) / np.float32(s))
    return np.ascontiguousarray((W * s).astype(np.float32)), inv_s


def _build(inv_s, mm_dt=MM_DT, enable_asserts=False, warm_mms=12):
    """Per-core Tile kernel: quant8 + [B_C,KN]@[KN,JM] + bias.

    sync ring:   x and W chunks interleaved (x_c before w_c), bias last;
                 plus 2 output chunks. scalar ring: 2 output chunks.
    ACT: quant pass 1 (x*inv_s+MAGIC); DVE: round+clip and the PSUM evicts
    (fused bias add). Warm matmuls keep the PE HAM gate open until w0 lands.
    x, W, bias, out are all fp16 (q codes and products exact; W/bias/out
    rounding ~1e-4 rel).
    """
    nc = bacc.Bacc(
        "TRN2", target_bir_lowering=False, debug=False,
        enable_asserts=enable_asserts, num_devices=N_CORES,
    )
    XC = 2                    # x DMA chunks
    WC = 8                    # w DMA chunks
    KPX = K_TILES // XC       # k-tiles per x chunk
    KPW = K_TILES // WC       # k-tiles per w chunk
    xt = nc.dram_tensor("xt", [P, K_TILES * B_C], mm_dt, kind="ExternalInput").ap()
    wt = nc.dram_tensor("wt", [KN, JM], mm_dt, kind="ExternalInput").ap()
    bi = nc.dram_tensor("bi", [JM], mm_dt, kind="ExternalInput").ap()
    out = nc.dram_tensor("out", [B_C, JM], mm_dt, kind="ExternalOutput").ap()

    xt_t = xt.rearrange("p (ko b) -> p ko b", b=B_C)
    wt_t = wt.rearrange("(ko p) j -> p ko j", p=P)
    out_t = out.rearrange("(bt p) j -> bt p j", p=P)

    with tile.TileContext(nc) as tc:
        with (
            tc.tile_pool(name="sb", bufs=1) as sb,
            tc.tile_pool(name="ps", bufs=1, space="PSUM") as ps,
        ):
            # input streams, interleaved on the sync ring; bias (only needed
            # at evict time) last
            x_sb = sb.tile([P, K_TILES, B_C], mm_dt, tag="x")
            w_sb = [sb.tile([P, KPW, JM], mm_dt, tag=f"w{c}", name=f"w{c}") for c in range(WC)]
            bias_sb = sb.tile([1, JM], mm_dt, tag="bias")
            nc.scalar.dma_start(bias_sb[:], bi[None, :])
            ones_sb = sb.tile([1, P], mm_dt, tag="ones")
            nc.gpsimd.memset(ones_sb[:], 1.0)
            # ring order: x0, w0, w1, x1, w2..w7 (x0/w0 land earliest)
            nc.sync.dma_start(x_sb[:, ts(0, KPX)], xt_t[:, ts(0, KPX)])
            nc.sync.dma_start(w_sb[0][:], wt_t[:, ts(0, KPW)])
            nc.sync.dma_start(w_sb[1][:], wt_t[:, ts(1, KPW)])
            nc.sync.dma_start(x_sb[:, ts(1, KPX)], xt_t[:, ts(1, KPX)])
            for c in range(2, WC):
                nc.sync.dma_start(w_sb[c][:], wt_t[:, ts(c, KPW)])

            # PE pre-warm on a zero tile (results never used): keeps the HAM
            # clock gate open from kernel start until w0 lands
            warm_psum = None
            if warm_mms:
                warm_sb = sb.tile([P, NH], mm_dt, tag="warm")
                nc.gpsimd.memset(warm_sb[:], 0.0)
                warm_psum = ps.tile([P, NH], mybir.dt.float32, tag="pswarm")
                for _ in range(warm_mms):
                    nc.tensor.matmul(
                        warm_psum[:], lhsT=warm_sb[:, :P], rhs=warm_sb[:],
                        start=True, stop=True,
                    )

            # quant pipeline (per x chunk): ACT scale+magic, DVE round+clip
            t_sb = sb.tile([P, K_TILES, B_C], mybir.dt.float32, tag="t")
            q_sb = sb.tile([P, K_TILES, B_C], mm_dt, tag="q")
            for c in range(XC):
                nc.scalar.activation(
                    t_sb[:, ts(c, KPX)], x_sb[:, ts(c, KPX)],
                    mybir.ActivationFunctionType.Copy,
                    bias=MAGIC, scale=inv_s,
                )
                nc.vector.tensor_scalar(
                    t_sb[:, ts(c, KPX)], t_sb[:, ts(c, KPX)], MAGIC, QMAX,
                    mybir.AluOpType.subtract, mybir.AluOpType.min,
                )
                nc.vector.tensor_scalar_max(
                    q_sb[:, ts(c, KPX)], t_sb[:, ts(c, KPX)], -QMAX,
                )

            psum = {
                (bt, nh): ps.tile([P, NH], mybir.dt.float32, tag=f"ps{bt}{nh}", name=f"ps{bt}{nh}")
                for bt in range(B_TILES) for nh in range(N_TILES)
            }
            # k-outer: PE tracks the W stream; all banks finish right after w_last
            for k in range(K_TILES):
                for bt in range(B_TILES):
                    for nh in range(N_TILES):
                        nc.tensor.matmul(
                            psum[(bt, nh)][:],
                            lhsT=q_sb[:, k, ts(bt, P)],
                            rhs=w_sb[k // KPW][:, k % KPW, ts(nh, NH)],
                            start=(k == 0),
                            stop=(k == K_TILES - 1),
                        )
                if k == K_TILES - 2:
                    # bias accumulation: outer product ones[128] x bias[512]
                    for bt in range(B_TILES):
                        for nh in range(N_TILES):
                            nc.tensor.matmul(
                                psum[(bt, nh)][:],
                                lhsT=ones_sb[:],
                                rhs=bias_sb[:, ts(nh, NH)],
                                start=False, stop=False,
                            )

            for bt in range(B_TILES):
                o_sb = sb.tile([P, JM], mm_dt, tag=f"o{bt}", name=f"o{bt}")
                nc.vector.tensor_copy(out=o_sb[:, ts(0, NH)], in_=psum[(bt, 0)][:])
                nc.sync.dma_start(out_t[bt][:, ts(0, NH)], o_sb[:, ts(0, NH)])
                nc.scalar.copy(out=o_sb[:, ts(1, NH)], in_=psum[(bt, 1)][:])
                nc.scalar.dma_start(out_t[bt][:, ts(1, NH)], o_sb[:, ts(1, NH)])

            if warm_mms:
                # keep the warm matmuls live (guard against DCE)
                sink = sb.tile([1, 1], mybir.dt.float32, tag="sink")
                nc.vector.tensor_copy(out=sink[:], in_=warm_psum[0:1, 0:1])

    nc.compile()
    return nc


def _prepare_inputs(x, Y_fp, Z_fp, A, bias, act_scale):
    W_s, inv_s = _fold_weights(Y_fp, Z_fp, A, act_scale)
    W_s = W_s.astype(np.float16)
    xT = np.asarray(x, dtype=np.float32).reshape(B_TOT, KN).T.astype(np.float16)
    bias16 = np.ascontiguousarray(np.asarray(bias, dtype=np.float16))
    in_maps = []
    for c in range(N_CORES):
        xc = xT[:, c * B_C:(c + 1) * B_C]                      # [KN, B_C]
        xc = np.ascontiguousarray(
            xc.reshape(K_TILES, P, B_C).transpose(1, 0, 2).reshape(P, K_TILES * B_C)
        )
        in_maps.append({"xt": xc, "wt": W_s, "bi": bias16})
    return in_maps, inv_s


def kernel_run(x, Y_fp, Z_fp, A, bias, act_scale, trace=False, **spmd_kwargs):
    """Build + run on 8 NeuronCores; returns (out, BassKernelResults)."""
    in_maps, inv_s = _prepare_inputs(x, Y_fp, Z_fp, A, bias, act_scale)
    nc = _build(inv_s)
    res = run_bass_kernel_spmd(
        nc, in_maps, core_ids=list(range(N_CORES)), trace=trace, **spmd_kwargs
    )
    out = np.concatenate([r["out"] for r in res.results], axis=0)  # [B_TOT, JM]
    out = out.astype(np.float32).reshape(x.shape[0], x.shape[1], JM).astype(x.dtype, copy=False)
    return out, res


def kernel(x, Y_fp, Z_fp, A, bias, act_scale):
    out, _ = kernel_run(x, Y_fp, Z_fp, A, bias, act_scale, trace=False)
    return out
